# revision 10
# baseline (speedup 1.0000x reference)
"""Causal self-attention head (softmax over the QUERY axis) on 8 trn2 cores.

Reference math (note the unusual softmax axis=-2, i.e. per key-column):
    q = x @ Wq; k = x @ Wk; v = x @ Wv            # [B,T,64]
    s[b,q,k] = (q . k) * 64**-0.5, masked to q >= k
    w[:, k]  = softmax over q of s[:, k]           # column softmax
    out[b,q,:] = sum_k w[q,k] v[k,:]

Because the softmax normalizes over q (the contraction axis of the second
matmul is k), the normalizer folds into a per-key scaling of v:
    out[q] = sum_{k<=q} exp(s[q,k]) * (r[k] * v[k]),  r[k] = 1/sum_{q>=k} exp(s[q,k])

Sharding: 8 cores = 4 batches x 2 "parities". Core (b, p) owns the key-column
blocks kb = 2i+p (i=0..15, 128 columns each) of batch b and produces a partial
output over all q; the host adds the two parity partials per batch.

To keep the SPMD program identical on all cores, parity-1 cores receive x^T
shifted left by 128 columns (zero-padded tail); their key blocks then sit at
the same compile-time offsets as parity-0's, and the host shifts their output
back by +128 q positions. Garbage q-columns from the zero pad are killed by a
per-core "tailmask" input.

Hardware notes:
- Each TPB instruction can carry at most ONE semaphore wait, so the program is
  structured so no instruction ever needs two cross-engine waits: causal masks
  are applied by accumulating triangular-count matmuls into the scores PSUM
  (PE-local), psum->sbuf copies and the v scaling run on the scalar engine,
  and tiny "absorber" matmuls observe each DMA completion before the real
  consumers run.
- exp runs on ACT with the 1/sqrt(64) folded into its free affine pre-scale
  and the per-column softmax sums produced by its fused accum_out.
"""

import os
import sys

import numpy as np

for _p in ("/opt/trn_rl_repo",):
    if _p not in sys.path:
        sys.path.insert(0, _p)

import concourse.bass as bass
import concourse.mybir as mybir
from concourse import bacc
from concourse.bass_utils import run_bass_kernel_spmd
from concourse.masks import make_identity
from concourse.tile import TileContext

B, T, CE, CH = 4, 4096, 1024, 64
P = 128
NB = 16          # key blocks per core (128 cols each)
NCHUNK = 8       # 512-col chunks covering T
SCALE = CH ** -0.5
NEG = -1e30
M0 = NEG / P     # per-unit mask magnitude for the triangular-count mask

F32 = mybir.dt.float32
BF16 = mybir.dt.bfloat16

N_CORES = 8

# Results of the last run (for test harnesses: exec_time_ns etc.)
LAST_RESULTS = None


def _build_program():
    # Bacc (not plain Bass): its compile() pipeline legalizes multi-semaphore
    # waits into EventSemaphore instructions and moves matmul waits onto
    # LDWEIGHTS — required by the 1-wait-per-instruction hardware encoding.
    nc = bacc.Bacc("TRN2", target_bir_lowering=False, debug=False)

    xT = nc.declare_dram_parameter("xT", [CE, T], BF16, isOutput=False)
    wq = nc.declare_dram_parameter("wq", [CE, CH], BF16, isOutput=False)
    wk = nc.declare_dram_parameter("wk", [CE, CH], BF16, isOutput=False)
    wv = nc.declare_dram_parameter("wv", [CE, CH], BF16, isOutput=False)
    tailmask = nc.declare_dram_parameter("tailmask", [P, P], BF16, isOutput=False)
    outT = nc.declare_dram_parameter("outT", [CH, T], F32, isOutput=True)

    with TileContext(nc) as tc:
        with (
            tc.tile_pool(name="consts", bufs=1) as consts,
            tc.tile_pool(name="qkv", bufs=1) as qkv,
            tc.tile_pool(name="w2p", bufs=1) as w2p,
            tc.tile_pool(name="xp", bufs=8) as xp,
            tc.tile_pool(name="pp", bufs=3, space="PSUM") as pp,
            tc.tile_pool(name="tp", bufs=1, space="PSUM") as tp,
            tc.tile_pool(name="sp", bufs=2, space="PSUM") as sp,
            tc.tile_pool(name="op", bufs=2, space="PSUM") as op,
        ):
            # ---- DMA'd constants ----
            wq_sb = consts.tile([P, CE // P, CH], BF16, tag="wq")
            wk_sb = consts.tile([P, CE // P, CH], BF16, tag="wk")
            wv_sb = consts.tile([P, CE // P, CH], BF16, tag="wv")
            nc.sync.dma_start(wq_sb[:], wq.rearrange("(o p) f -> p o f", p=P))
            nc.sync.dma_start(wk_sb[:], wk.rearrange("(o p) f -> p o f", p=P))
            nc.sync.dma_start(wv_sb[:], wv.rearrange("(o p) f -> p o f", p=P))

            tmask = consts.tile([P, P], BF16, tag="tmask")
            nc.sync.dma_start(tmask[:], tailmask[:])

            # ---- gpsimd-built constants ----
            # Atri[ch, p] = 1 if ch < p else 0; Bneg[ch, c] = M0 if c <= ch else 0
            # => (Atri^T @ Bneg)[p, c] = M0 * max(0, p - c): the causal mask.
            atri = consts.tile([P, P], BF16, tag="atri")
            nc.gpsimd.memset(atri[:], 1.0)
            nc.gpsimd.affine_select(
                out=atri[:],
                in_=atri[:],
                compare_op=mybir.AluOpType.is_ge,
                fill=0.0,
                base=-1,
                pattern=[[1, P]],
                channel_multiplier=-1,
            )
            bneg = consts.tile([P, 2 * P], BF16, tag="bneg")
            nc.gpsimd.memset(bneg[:], M0)
            nc.gpsimd.affine_select(
                out=bneg[:],
                in_=bneg[:],
                compare_op=mybir.AluOpType.is_ge,
                fill=0.0,
                base=0,
                pattern=[[-1, 2 * P]],
                channel_multiplier=1,
            )
            ones = consts.tile([P, P], BF16, tag="ones")
            nc.gpsimd.memset(ones[:], 1.0)
            # ident built LAST so one PE wait on it covers all gpsimd consts
            ident = consts.tile([CH, CH], BF16, tag="ident")
            make_identity(nc, ident[:])

            # ---- persistent activations ----
            qT = qkv.tile([CH, T], BF16, tag="qT")
            kTl = qkv.tile([CH, NB * P], BF16, tag="kTl")
            vT = qkv.tile([CH, NB * P], BF16, tag="vT")
            vnat = qkv.tile([P, NB, CH], BF16, tag="vnat")
            stats = qkv.tile([P, NB, 8], F32, tag="stats")
            ssum = qkv.tile([P, NB], F32, tag="ssum")
            rr = qkv.tile([P, NB], F32, tag="rr")
            outsb = qkv.tile([CH, T], F32, tag="outsb")

            w2 = [
                w2p.tile([P, T - 256 * i], BF16, tag=f"w2_{i}", name=f"w2_{i}")
                for i in range(NB)
            ]

            # PE absorber: observe all gpsimd consts (ident is last of them)
            dscr = op.tile([CH, 512], F32, tag="po", name="dscr0")
            nc.tensor.matmul(
                dscr[0:1, 0:1], ident[:, 0:1], ident[:, 0:1], start=True, stop=True
            )
            # absorber for the tailmask DMA
            dscr = op.tile([CH, 512], F32, tag="po", name="dscr1")
            nc.tensor.matmul(
                dscr[0:1, 0:1], tmask[0:CH, 0:1], tmask[0:CH, 0:1],
                start=True, stop=True,
            )

            # ================= Phase A: projections =================
            for j in range(NCHUNK):
                xtile = xp.tile([P, CE // P, 512], BF16, tag="xtile")
                nc.sync.dma_start(
                    xtile[:],
                    xT[:, 512 * j : 512 * (j + 1)].rearrange(
                        "(o p) f -> p o f", p=P
                    ),
                )
                # absorber: put this chunk's DMA-queue wait on a throwaway MM
                dscr = op.tile([CH, 512], F32, tag="po", name=f"dscrx{j}")
                nc.tensor.matmul(
                    dscr[0:1, 0:1],
                    xtile[:, 0, 0:1],
                    xtile[:, 0, 0:1],
                    start=True,
                    stop=True,
                )

                # q projection: full 512 columns
                psq = pp.tile([CH, 512], F32, tag="proj")
                for s in range(CE // P):
                    nc.tensor.matmul(
                        psq[:],
                        wq_sb[:, s, :],
                        xtile[:, s, :],
                        start=(s == 0),
                        stop=(s == CE // P - 1),
                    )
                nc.scalar.copy(qT[:, 512 * j : 512 * (j + 1)], psq[:])

                # k/v projections: the 2 key blocks of this chunk sit at
                # column offsets 0 and 256 (thanks to the parity shift).
                psk = pp.tile([CH, 512], F32, tag="proj")
                psv = pp.tile([CH, 512], F32, tag="proj")
                for s in range(CE // P):
                    for r, off in enumerate((0, 256)):
                        nc.tensor.matmul(
                            psk[:, P * r : P * (r + 1)],
                            wk_sb[:, s, :],
                            xtile[:, s, off : off + P],
                            start=(s == 0 and r == 0),
                            stop=(s == CE // P - 1 and r == 1),
                        )
                for s in range(CE // P):
                    for r, off in enumerate((0, 256)):
                        nc.tensor.matmul(
                            psv[:, P * r : P * (r + 1)],
                            wv_sb[:, s, :],
                            xtile[:, s, off : off + P],
                            start=(s == 0 and r == 0),
                            stop=(s == CE // P - 1 and r == 1),
                        )
                nc.scalar.copy(kTl[:, 256 * j : 256 * (j + 1)], psk[:, 0:256])
                nc.scalar.copy(vT[:, 256 * j : 256 * (j + 1)], psv[:, 0:256])

            # v^T -> v natural [128 key rows, 64 ch] per block
            for i in range(NB):
                tps = tp.tile([P, CH], BF16, tag="tps")
                nc.tensor.transpose(tps[:], vT[:, P * i : P * (i + 1)], ident[:])
                nc.scalar.copy(vnat[:, i, :], tps[:])

            # ================= Phase B: scores + column softmax =================
            for i in range(NB):
                qlo = 256 * i
                L = T - qlo
                nch = (L + 511) // 512
                for c in range(nch):
                    w = min(512, L - 512 * c)
                    sc = sp.tile([P, 512], F32, tag="sc")
                    last = (c != 0) and (c != nch - 1)
                    nc.tensor.matmul(
                        sc[:, :w],
                        kTl[:, P * i : P * (i + 1)],
                        qT[:, qlo + 512 * c : qlo + 512 * c + w],
                        start=True,
                        stop=last,
                    )
                    if c == 0:
                        # causal mask: += M0 * max(0, p - col)
                        nc.tensor.matmul(
                            sc[:, 0:256],
                            atri[:],
                            bneg[:],
                            start=False,
                            stop=(nch != 1),
                        )
                    if c == nch - 1:
                        # zero-pad tail kill: += sum_ch tailmask[ch, col]
                        nc.tensor.matmul(
                            sc[:, w - P : w],
                            ones[:],
                            tmask[:],
                            start=False,
                            stop=True,
                        )
                    nc.scalar.activation(
                        w2[i][:, 512 * c : 512 * c + w],
                        sc[:, :w],
                        mybir.ActivationFunctionType.Exp,
                        scale=SCALE,
                        accum_out=stats[:, i, c : c + 1],
                    )
                nc.vector.reduce_sum(
                    ssum[:, i : i + 1], stats[:, i, 0:nch], axis=mybir.AxisListType.X
                )
                nc.vector.reciprocal(rr[:, i : i + 1], ssum[:, i : i + 1])
                nc.scalar.activation(
                    vnat[:, i, :],
                    vnat[:, i, :],
                    mybir.ActivationFunctionType.Copy,
                    bias=0.0,
                    scale=rr[:, i : i + 1],
                )

            # ================= Phase C: out^T = sum_i vr_i^T @ w2_i =================
            for c in range(NCHUNK):
                po = op.tile([CH, 512], F32, tag="po", name=f"po{c}")
                ilast = min(2 * c + 1, NB - 1)
                for i in range(ilast + 1):
                    off = 512 * c - 256 * i
                    if off >= 0:
                        nc.tensor.matmul(
                            po[:],
                            vnat[:, i, :],
                            w2[i][:, off : off + 512],
                            start=(i == 0),
                            stop=(i == ilast),
                        )
                    else:
                        nc.tensor.matmul(
                            po[:, 256:512],
                            vnat[:, i, :],
                            w2[i][:, 0:256],
                            start=False,
                            stop=(i == ilast),
                        )
                nc.scalar.copy(outsb[:, 512 * c : 512 * (c + 1)], po[:])

            nc.sync.dma_start(outT[:], outsb[:])

    return nc


_PROGRAM = None


def _get_program():
    global _PROGRAM
    if _PROGRAM is None:
        nc = _build_program()
        nc.finalize()
        _PROGRAM = nc
    return _PROGRAM


def kernel(x, Wk, Wq, Wv, trace=False, trace_cores=None):
    global LAST_RESULTS
    x = np.asarray(x)
    Wk = np.asarray(Wk)
    Wq = np.asarray(Wq)
    Wv = np.asarray(Wv)

    import ml_dtypes

    bf = ml_dtypes.bfloat16
    wq_b = Wq.astype(bf)
    wk_b = Wk.astype(bf)
    wv_b = Wv.astype(bf)

    zeros_mask = np.zeros((P, P), bf)
    neg_mask = np.full((P, P), NEG / P, bf)

    in_maps = []
    for c in range(N_CORES):
        b, parity = c // 2, c % 2
        xTb = np.ascontiguousarray(x[b].T).astype(bf)  # [CE, T]
        if parity:
            xTb = np.concatenate([xTb[:, P:], np.zeros((CE, P), bf)], axis=1)
        in_maps.append(
            {
                "xT": np.ascontiguousarray(xTb),
                "wq": wq_b,
                "wk": wk_b,
                "wv": wv_b,
                "tailmask": neg_mask if parity else zeros_mask,
            }
        )

    nc = _get_program()
    res = run_bass_kernel_spmd(
        nc,
        in_maps,
        list(range(N_CORES)),
        trace=trace,
        **({"trace_cores": trace_cores} if trace_cores is not None else {}),
    )
    LAST_RESULTS = res

    out = np.zeros((B, T, CH), np.float32)
    for c in range(N_CORES):
        b, parity = c // 2, c % 2
        oT = np.asarray(res.results[c]["outT"], np.float32)  # [CH, T]
        if parity:
            # core's column t corresponds to q = t + 128
            out[b, P:, :] += oT[:, : T - P].T
        else:
            out[b] += oT.T
    return out
